# revision 17
# baseline (speedup 1.0000x reference)
"""MoE routing kernel for Trainium2 (8 NeuronCores, expert-parallel).

Problem: top-2-of-8 expert MLP with squared-ReLU, d_model=1024, d_ff=1024,
N=8192 tokens. Strategy: the router (softmax + top-2, ~0.2% of FLOPs) runs
on host in float64; tokens are dispatched on host (gather + sqrt(combine-
weight) scaling — relu(sqrt(w)*z)^2 == w*relu(z)^2, so the combine weight
folds into the input and the device kernel is a plain 2-layer MLP in
float32r). Load balance: each expert's tokens are split across two cores;
every core serves one "big" and one "small" expert half (capacities CA/CB
identical across cores, so the single SPMD program fits all cores and the
per-core column count tracks the mean batch rather than the max expert).
Host scatter-adds the per-core outputs.
"""

import sys

if "/opt/trn_rl_repo" not in sys.path:
    sys.path.insert(0, "/opt/trn_rl_repo")

import numpy as np

import bass_rust
import concourse.bass as bass
import concourse.tile as tile
import concourse.tile_utils as tile_utils
from concourse import mybir
from concourse.bass_utils import run_bass_kernel_spmd
from concourse.vector_clock import ScopedClock

NUM_EXPERTS = 8
TOP_K = 2
D_MODEL = 1024
D_FF = 1024
N_CORES = 8
KC = D_MODEL // 128
FT = D_FF // 128
DT = D_MODEL // 128

# Cayman has 208 KiB/partition usable; the stock constant leaves 16 KiB idle.
tile_utils.max_sbuf_usage = 208 * 1024

# ---------------------------------------------------------------------------
# Compat: this container's walrus rejects instructions carrying more than one
# sem wait ("Too many sync wait commands"). Replace the TileContext final
# drain with single-wait SP nops, and post-process the module so every
# instruction carries at most one (monotonic) wait.
# ---------------------------------------------------------------------------


def _patched_drain_and_barrier(self, tick_clock, wait_clock):
    probe = self.nc.sync.nop(nofuse=True)
    wait_clock.add_sem_waits(probe.ins, ScopedClock({None: tick_clock.global_clock}))
    si = probe.ins.sync_info
    waits = list(si.on_wait) if si is not None else []
    updates = list(si.on_update) if si is not None else []
    if len(waits) > 1:
        probe.ins.sync_info = bass_rust.SyncInfo(on_wait=[waits[0]], on_update=updates)
        for w in waits[1:]:
            extra = self.nc.sync.nop(nofuse=True)
            extra.ins.sync_info = bass_rust.SyncInfo(on_wait=[w], on_update=[])
    self.nc.sync.drain()
    self.nc.all_engine_barrier()
    assert self.sems is not None
    popped = self.nc._tile_sem_poison_stack.pop()
    assert popped is self._sem_poison
    self.nc.clear_and_free_semaphores(list(self.sems.allocated().values()))
    self.nc.all_engine_barrier()


tile.TileContext._drain_and_barrier = _patched_drain_and_barrier


def split_excess_waits(nc, limit=1):
    for fn in nc.m.functions:
        for bb in fn.blocks:
            il = bb.instructions
            i = 0
            while i < len(il):
                inst = il[i]
                si = inst.sync_info
                if si is not None and len(si.on_wait) > limit:
                    waits = list(si.on_wait)
                    movable = [w for w in waits if "ge" in (w.wait_mode or "")]
                    pinned = [w for w in waits if w not in movable]
                    keep_n = max(0, limit - len(pinned))
                    if keep_n:
                        keep = pinned + movable[len(movable) - keep_n :]
                        extra = movable[: len(movable) - keep_n]
                    else:
                        keep, extra = pinned, movable
                    if not extra:
                        i += 1
                        continue
                    nops = []
                    for w in extra:
                        nop = mybir.InstNoOp(
                            name=nc.get_next_instruction_name(), ins=[], outs=[]
                        )
                        nop.engine = inst.engine
                        nop.sync_info = bass_rust.SyncInfo(on_wait=[w], on_update=[])
                        nops.append(nop)
                    inst.sync_info = bass_rust.SyncInfo(
                        on_wait=keep, on_update=list(si.on_update)
                    )
                    for j, nop in enumerate(nops):
                        il.insert(i + j, nop)
                    i += len(nops)
                i += 1


# ---------------------------------------------------------------------------
# Capacities and token blocks. Every block is >= 256 columns (full f32r
# matmul rate) and <= 320 (keeps four resident weight matrices + working
# tiles inside SBUF).
# ---------------------------------------------------------------------------

F32R = mybir.dt.float32r
F32 = mybir.dt.float32


def _seg_cap(n):
    """Round half-segment length up to a capacity decomposable into blocks
    of 256..320 columns."""
    cap = max(256, -(-n // 128) * 128)
    if cap == 384:
        cap = 512
    return cap


def _seg_sizes(cap):
    """Decompose cap (256 or any multiple of 128 >= 512) into blocks of
    256..320 columns."""
    if cap == 256:
        return [256]
    n320 = 0
    while (cap - 320 * n320) % 256 != 0:
        n320 += 1
        assert n320 <= 4 and cap - 320 * n320 >= 0
    return [320] * n320 + [256] * ((cap - 320 * n320) // 256)


def _blocks(ca, cb):
    out = []
    t = 0
    for seg, cap in ((0, ca), (1, cb)):
        for tb in _seg_sizes(cap):
            out.append((t, tb, seg))
            t += tb
    return out


def build_program(ca, cb):
    cap = ca + cb
    nc = bass.Bass("TRN2", target_bir_lowering=False, debug=False, num_devices=N_CORES)
    # xP: host-packed [128, KC*cap]; token block (t0,tb) occupies columns
    # [KC*t0, KC*(t0+tb)) laid out [p, (kc t)] — one contiguous DMA per block.
    # yP: same packing for the output, [p, (dt t)].
    # w*: host-prepacked so row block m holds [p, (kc c)] — the [128,128]
    # lhsT tiles for output-tile group m are one contiguous 512 KiB chunk,
    # streamed in consumption order (w1a, w2a, w1b, w2b).
    xP = nc.declare_dram_parameter("xP", [128, KC * cap], F32R, isOutput=False)
    w1a = nc.declare_dram_parameter("w1a", [D_MODEL, D_FF], F32R, isOutput=False)
    w2a = nc.declare_dram_parameter("w2a", [D_FF, D_MODEL], F32R, isOutput=False)
    w1b = nc.declare_dram_parameter("w1b", [D_MODEL, D_FF], F32R, isOutput=False)
    w2b = nc.declare_dram_parameter("w2b", [D_FF, D_MODEL], F32R, isOutput=False)
    yP = nc.declare_dram_parameter("yP", [128, DT * cap], F32, isOutput=True)

    w_drams = [
        (
            w1a.rearrange("(m p) x -> m p x", p=128),
            w2a.rearrange("(m p) x -> m p x", p=128),
        ),
        (
            w1b.rearrange("(m p) x -> m p x", p=128),
            w2b.rearrange("(m p) x -> m p x", p=128),
        ),
    ]

    with tile.TileContext(nc) as tc:
        with (
            tc.tile_pool(name="wpool", bufs=1) as wpool,
            tc.tile_pool(name="xpool", bufs=3) as xpool,
            tc.tile_pool(name="mpool", bufs=2) as mpool,
            tc.tile_pool(name="tpool", bufs=2) as tpool,
            tc.tile_pool(name="opool", bufs=2) as opool,
            tc.tile_pool(name="psum", bufs=2, space="PSUM") as psum_pool,
        ):
            w_sbs = []
            for seg in (0, 1):
                w1_sb = wpool.tile([128, FT * D_MODEL], F32R, tag=f"w1{seg}")
                w2_sb = wpool.tile([128, DT * D_FF], F32R, tag=f"w2{seg}")
                w_sbs.append((w1_sb, w2_sb))
            # Weight DMAs on the ACT HWDGE queue in consumption order; x and
            # output stores ride the SP HWDGE / Pool SWDGE queues.
            for seg in (0, 1):
                for which in (0, 1):
                    sb = w_sbs[seg][which]
                    dram = w_drams[seg][which]
                    for m in range(8):
                        nc.scalar.dma_start(sb[:, m * 1024 : (m + 1) * 1024], dram[m])

            blocks = _blocks(ca, cb)
            for bi, (t0, tb, seg) in enumerate(blocks):
                w1_sb, w2_sb = w_sbs[seg]
                x_sb = xpool.tile([128, KC * tb], F32R, tag="x")
                nc.sync.dma_start(x_sb[:], xP[:, KC * t0 : KC * (t0 + tb)])
                mid_sb = mpool.tile([128, FT * tb], F32R, tag="mid")
                for ft in range(FT):
                    ps = psum_pool.tile([128, tb], F32, tag="ps")
                    for kc in range(KC):
                        nc.tensor.matmul(
                            ps[:],
                            w1_sb[
                                :,
                                ft * D_MODEL + kc * 128 : ft * D_MODEL + kc * 128 + 128,
                            ],
                            x_sb[:, kc * tb : (kc + 1) * tb],
                            start=(kc == 0),
                            stop=(kc == KC - 1),
                        )
                    tmp = tpool.tile([128, tb], F32, tag="tmp")
                    nc.scalar.activation(
                        tmp[:], ps[:], mybir.ActivationFunctionType.Relu
                    )
                    nc.vector.tensor_mul(
                        mid_sb[:, ft * tb : (ft + 1) * tb], tmp[:], tmp[:]
                    )
                o_sb = opool.tile([128, DT * tb], F32, tag="o")
                for dt_ in range(DT):
                    ps2 = psum_pool.tile([128, tb], F32, tag="ps2")
                    for fc in range(FT):
                        nc.tensor.matmul(
                            ps2[:],
                            w2_sb[
                                :, dt_ * D_FF + fc * 128 : dt_ * D_FF + fc * 128 + 128
                            ],
                            mid_sb[:, fc * tb : (fc + 1) * tb],
                            start=(fc == 0),
                            stop=(fc == FT - 1),
                        )
                    nc.scalar.copy(o_sb[:, dt_ * tb : (dt_ + 1) * tb], ps2[:])
                o_eng = nc.sync if bi >= len(blocks) - 2 else nc.gpsimd
                o_eng.dma_start(yP[:, DT * t0 : DT * (t0 + tb)], o_sb[:])

    split_excess_waits(nc, limit=1)
    return nc


_PROGRAM_CACHE = {}


def _get_program(ca, cb):
    if (ca, cb) not in _PROGRAM_CACHE:
        _PROGRAM_CACHE[(ca, cb)] = build_program(ca, cb)
    return _PROGRAM_CACHE[(ca, cb)]


# ---------------------------------------------------------------------------
# Host side: routing, dispatch, combine.
# ---------------------------------------------------------------------------


def _pack_blocked(aT, cap, blocks):
    """[1024, cap] feature-major -> [128, 8*cap], each token block laid out
    [p, (g t)] so the device moves one contiguous chunk per block."""
    g = aT.shape[0] // 128
    out = np.empty((128, g * cap), aT.dtype)
    for t0, tb, _seg in blocks:
        out[:, g * t0 : g * (t0 + tb)] = (
            aT[:, t0 : t0 + tb]
            .reshape(g, 128, tb)
            .transpose(1, 0, 2)
            .reshape(128, g * tb)
        )
    return out


def _unpack_blocked(aP, cap, blocks):
    g = aP.shape[1] // cap
    out = np.empty((g * 128, cap), aP.dtype)
    for t0, tb, _seg in blocks:
        blk = aP[:, g * t0 : g * (t0 + tb)].reshape(128, g, tb)
        out[:, t0 : t0 + tb] = blk.transpose(1, 0, 2).reshape(g * 128, tb)
    return out


def _prep_weight(w):
    k, m = w.shape
    return np.ascontiguousarray(
        w.reshape(k // 128, 128, m // 128, 128).transpose(2, 1, 0, 3).reshape(m, k),
        dtype=np.float32,
    )


def kernel(x, Wr, W1, W2, _trace=False):
    x = np.asarray(x)
    Wr = np.asarray(Wr)
    W1 = np.asarray(W1)
    W2 = np.asarray(W2)
    B, T, C = x.shape
    N = B * T
    xf = np.ascontiguousarray(x.reshape(N, C), dtype=np.float32)

    # Router in float64 (matches jax f32 top_k selections; verified).
    logits = xf.astype(np.float64) @ Wr.astype(np.float64)
    logits -= logits.max(axis=-1, keepdims=True)
    p = np.exp(logits)
    p /= p.sum(axis=-1, keepdims=True)
    idx = np.argsort(-p, axis=-1, kind="stable")[:, :TOP_K]  # [N, K]
    wts = np.take_along_axis(p, idx, axis=-1)  # [N, K]

    # Dispatch list sorted by expert.
    flat_e = idx.ravel()
    order = np.argsort(flat_e, kind="stable")
    tok_of_pair = np.repeat(np.arange(N), TOP_K)[order]
    w_of_pair = wts.ravel()[order]
    counts = np.bincount(flat_e, minlength=NUM_EXPERTS)
    starts = np.concatenate([[0], np.cumsum(counts)[:-1]])

    # Pair big experts with small ones; each expert's tokens split over two
    # cores. Core c serves half (c // 4) of big expert bigs[c % 4] and of
    # small expert smalls[c % 4].
    by_count = np.argsort(-counts, kind="stable")
    bigs, smalls = by_count[:4], by_count[7:3:-1]
    ca = _seg_cap(int(-(-int(counts[bigs[0]]) // 2)))
    cb = _seg_cap(int(-(-int(counts[smalls].max()) // 2)))
    cap = ca + cb
    blocks = _blocks(ca, cb)

    def expert_half(e, h):
        s, c = int(starts[e]), int(counts[e])
        m = -(-c // 2)
        sl = slice(s, s + m) if h == 0 else slice(s + m, s + c)
        return tok_of_pair[sl], w_of_pair[sl]

    in_maps = []
    core_toks = []
    for c_id in range(N_CORES):
        pi, h = c_id % 4, c_id // 4
        eb, es = int(bigs[pi]), int(smalls[pi])
        xTe = np.zeros((C, cap), np.float32)
        toks_ab = []
        for e, off in ((eb, 0), (es, ca)):
            toks, ws = expert_half(e, h)
            xg = xf[toks] * np.sqrt(ws.astype(np.float32))[:, None]
            xTe[:, off : off + len(toks)] = xg.T
            toks_ab.append((toks, off))
        core_toks.append(toks_ab)
        in_maps.append(
            {
                "xP": _pack_blocked(xTe, cap, blocks),
                "w1a": _prep_weight(W1[eb]),
                "w2a": _prep_weight(W2[eb]),
                "w1b": _prep_weight(W1[es]),
                "w2b": _prep_weight(W2[es]),
            }
        )

    nc = _get_program(ca, cb)
    res = run_bass_kernel_spmd(nc, in_maps, core_ids=list(range(N_CORES)), trace=_trace)

    out = np.zeros((N, C), np.float32)
    for c_id in range(N_CORES):
        yT = _unpack_blocked(res.results[c_id]["yP"], cap, blocks)
        for toks, off in core_toks[c_id]:
            if len(toks):
                out[toks] += yT[:, off : off + len(toks)].T
    if _trace:
        kernel._last_exec_time_ns = res.exec_time_ns
    return out.reshape(B, T, C)


# revision 19
# speedup vs baseline: 1.1789x; 1.1789x over previous
"""MoE routing kernel for Trainium2 (8 NeuronCores, expert-parallel).

Problem: top-2-of-8 expert MLP with squared-ReLU, d_model=1024, d_ff=1024,
N=8192 tokens. Strategy: the router (softmax + top-2, ~0.2% of FLOPs) runs
on host in float64; tokens are dispatched on host (gather + sqrt(combine-
weight) scaling — relu(sqrt(w)*z)^2 == w*relu(z)^2, so the combine weight
folds into the input and the device kernel is a plain 2-layer MLP in
float32r). Load balance: each expert's tokens are split across two cores;
every core serves one "big" and one "small" expert half (capacities CA/CB
identical across cores, so the single SPMD program fits all cores and the
per-core column count tracks the mean batch rather than the max expert).
Host scatter-adds the per-core outputs.
"""

import sys

if "/opt/trn_rl_repo" not in sys.path:
    sys.path.insert(0, "/opt/trn_rl_repo")

import numpy as np

import bass_rust
import concourse.bass as bass
import concourse.tile as tile
import concourse.tile_utils as tile_utils
from concourse import mybir
from concourse.bass_utils import run_bass_kernel_spmd
from concourse.vector_clock import ScopedClock

NUM_EXPERTS = 8
TOP_K = 2
D_MODEL = 1024
D_FF = 1024
N_CORES = 8
KC = D_MODEL // 128
FT = D_FF // 128
DT = D_MODEL // 128

# Cayman has 208 KiB/partition usable; the stock constant leaves 16 KiB idle.
tile_utils.max_sbuf_usage = 208 * 1024

# ---------------------------------------------------------------------------
# Compat: this container's walrus rejects instructions carrying more than one
# sem wait ("Too many sync wait commands"). Replace the TileContext final
# drain with single-wait SP nops, and post-process the module so every
# instruction carries at most one (monotonic) wait.
# ---------------------------------------------------------------------------


def _patched_drain_and_barrier(self, tick_clock, wait_clock):
    probe = self.nc.sync.nop(nofuse=True)
    wait_clock.add_sem_waits(probe.ins, ScopedClock({None: tick_clock.global_clock}))
    si = probe.ins.sync_info
    waits = list(si.on_wait) if si is not None else []
    updates = list(si.on_update) if si is not None else []
    if len(waits) > 1:
        probe.ins.sync_info = bass_rust.SyncInfo(on_wait=[waits[0]], on_update=updates)
        for w in waits[1:]:
            extra = self.nc.sync.nop(nofuse=True)
            extra.ins.sync_info = bass_rust.SyncInfo(on_wait=[w], on_update=[])
    self.nc.sync.drain()
    self.nc.all_engine_barrier()
    assert self.sems is not None
    popped = self.nc._tile_sem_poison_stack.pop()
    assert popped is self._sem_poison
    self.nc.clear_and_free_semaphores(list(self.sems.allocated().values()))
    self.nc.all_engine_barrier()


tile.TileContext._drain_and_barrier = _patched_drain_and_barrier


def split_excess_waits(nc, limit=1):
    for fn in nc.m.functions:
        for bb in fn.blocks:
            il = bb.instructions
            i = 0
            while i < len(il):
                inst = il[i]
                si = inst.sync_info
                if si is not None and len(si.on_wait) > limit:
                    waits = list(si.on_wait)
                    movable = [w for w in waits if "ge" in (w.wait_mode or "")]
                    pinned = [w for w in waits if w not in movable]
                    keep_n = max(0, limit - len(pinned))
                    if keep_n:
                        keep = pinned + movable[len(movable) - keep_n :]
                        extra = movable[: len(movable) - keep_n]
                    else:
                        keep, extra = pinned, movable
                    if not extra:
                        i += 1
                        continue
                    nops = []
                    for w in extra:
                        nop = mybir.InstNoOp(
                            name=nc.get_next_instruction_name(), ins=[], outs=[]
                        )
                        nop.engine = inst.engine
                        nop.sync_info = bass_rust.SyncInfo(on_wait=[w], on_update=[])
                        nops.append(nop)
                    inst.sync_info = bass_rust.SyncInfo(
                        on_wait=keep, on_update=list(si.on_update)
                    )
                    for j, nop in enumerate(nops):
                        il.insert(i + j, nop)
                    i += len(nops)
                i += 1


# ---------------------------------------------------------------------------
# Capacities and token blocks. Every block is >= 256 columns (full f32r
# matmul rate) and <= 320 (keeps four resident weight matrices + working
# tiles inside SBUF).
# ---------------------------------------------------------------------------

F32R = mybir.dt.float32r
F32 = mybir.dt.float32


def _token_blocks(cap):
    """Blocks >= 256 cols (full f32r rate). Two small lead-in blocks so the
    first dependencies are tiny, 512 steady state, small tail for fast
    drain."""
    assert cap % 128 == 0 and cap >= 256
    sizes = []
    rem = cap
    for lead in (256, 256):
        if rem - lead >= 256 or rem == lead:
            sizes.append(lead)
            rem -= lead
        if rem == 0:
            break
    while rem > 768:
        sizes.append(512)
        rem -= 512
    if rem:
        if rem in (256, 384, 512):
            sizes.append(rem)
        else:  # 640, 768
            sizes.extend([rem - 256, 256])
    blocks, t = [], 0
    for tb in sizes:
        blocks.append((t, tb))
        t += tb
    assert t == cap, (cap, sizes)
    return blocks


def _chunks(c0, c1, step):
    out = []
    while c0 < c1:
        out.append((c0, min(c0 + step, c1)))
        c0 = out[-1][1]
    return out


def build_program(cap):
    nc = bass.Bass("TRN2", target_bir_lowering=False, debug=False, num_devices=N_CORES)
    # xP: host-packed [128, KC*cap]; token block (t0,tb) occupies columns
    # [KC*t0, KC*(t0+tb)) laid out [p, (kc t)]. yP likewise [p, (dt t)].
    # w1/w2 host-prepacked so row block m holds [p, (kc c)] — consumption
    # order, one contiguous 512 KiB chunk per output-tile group.
    xP = nc.declare_dram_parameter("xP", [128, KC * cap], F32R, isOutput=False)
    w1 = nc.declare_dram_parameter("w1", [D_MODEL, D_FF], F32R, isOutput=False)
    w2 = nc.declare_dram_parameter("w2", [D_FF, D_MODEL], F32R, isOutput=False)
    yP = nc.declare_dram_parameter("yP", [128, DT * cap], F32, isOutput=True)

    w1_r = w1.rearrange("(m p) x -> m p x", p=128)
    w2_r = w2.rearrange("(m p) x -> m p x", p=128)

    blocks = _token_blocks(cap)
    nb = len(blocks)

    with tile.TileContext(nc) as tc:
        with (
            tc.tile_pool(name="wpool", bufs=1) as wpool,
            tc.tile_pool(name="xpool", bufs=4) as xpool,
            tc.tile_pool(name="mpool", bufs=2) as mpool,
            tc.tile_pool(name="tpool", bufs=2) as tpool,
            tc.tile_pool(name="opool", bufs=2) as opool,
            tc.tile_pool(name="psum", bufs=2, space="PSUM") as psum_pool,
        ):
            w1_sb = wpool.tile([128, FT * D_MODEL], F32R, tag="w1")
            w2_sb = wpool.tile([128, DT * D_FF], F32R, tag="w2")
            # DMAs are issued in 4 KiB-line chunks — the HW DGE sustains
            # ~260 GB/s with 4 KiB packets vs ~110-160 GB/s with wider rows.
            for m in range(FT):
                nc.scalar.dma_start(w1_sb[:, m * 1024 : (m + 1) * 1024], w1_r[m])
            for m in range(DT):
                nc.scalar.dma_start(w2_sb[:, m * 1024 : (m + 1) * 1024], w2_r[m])

            for bi, (t0, tb) in enumerate(blocks):
                x_sb = xpool.tile([128, KC * tb], F32R, tag="x")
                # First three x blocks ride the SP queue (starts immediately);
                # later ones use the ACT queue, which is idle once the 8.4 MB
                # of weights has streamed.
                x_eng = nc.sync if bi < 3 else nc.scalar
                for c0, c1 in _chunks(0, KC * tb, 1024):
                    x_eng.dma_start(
                        x_sb[:, c0:c1], xP[:, KC * t0 + c0 : KC * t0 + c1]
                    )
                mid_sb = mpool.tile([128, FT * tb], F32R, tag="mid")
                for ft in range(FT):
                    ps = psum_pool.tile([128, tb], F32, tag="ps")
                    for kc in range(KC):
                        nc.tensor.matmul(
                            ps[:],
                            w1_sb[
                                :,
                                ft * D_MODEL + kc * 128 : ft * D_MODEL + kc * 128 + 128,
                            ],
                            x_sb[:, kc * tb : (kc + 1) * tb],
                            start=(kc == 0),
                            stop=(kc == KC - 1),
                        )
                    tmp = tpool.tile([128, tb], F32, tag="tmp")
                    nc.scalar.activation(
                        tmp[:], ps[:], mybir.ActivationFunctionType.Relu
                    )
                    nc.vector.tensor_mul(
                        mid_sb[:, ft * tb : (ft + 1) * tb], tmp[:], tmp[:]
                    )
                o_sb = opool.tile([128, DT * tb], F32, tag="o")
                for dt_ in range(DT):
                    ps2 = psum_pool.tile([128, tb], F32, tag="ps2")
                    for fc in range(FT):
                        nc.tensor.matmul(
                            ps2[:],
                            w2_sb[
                                :, dt_ * D_FF + fc * 128 : dt_ * D_FF + fc * 128 + 128
                            ],
                            mid_sb[:, fc * tb : (fc + 1) * tb],
                            start=(fc == 0),
                            stop=(fc == FT - 1),
                        )
                    nc.scalar.copy(o_sb[:, dt_ * tb : (dt_ + 1) * tb], ps2[:])
                # Outputs drain on the software DGE mid-kernel; the last two
                # blocks use the (by then idle) SP queue so the tail flushes
                # fast.
                o_eng = nc.sync if bi >= nb - 2 else nc.gpsimd
                for c0, c1 in _chunks(0, DT * tb, 1024):
                    o_eng.dma_start(
                        yP[:, DT * t0 + c0 : DT * t0 + c1], o_sb[:, c0:c1]
                    )

    split_excess_waits(nc, limit=1)
    return nc


_PROGRAM_CACHE = {}


def _get_program(cap):
    if cap not in _PROGRAM_CACHE:
        _PROGRAM_CACHE[cap] = build_program(cap)
    return _PROGRAM_CACHE[cap]


# ---------------------------------------------------------------------------
# Host side: routing, dispatch, combine.
# ---------------------------------------------------------------------------


def _pack_blocked(aT, cap, blocks):
    """[1024, cap] feature-major -> [128, 8*cap], each token block laid out
    [p, (g t)] so the device moves one contiguous chunk per block."""
    g = aT.shape[0] // 128
    out = np.empty((128, g * cap), aT.dtype)
    for t0, tb in blocks:
        out[:, g * t0 : g * (t0 + tb)] = (
            aT[:, t0 : t0 + tb]
            .reshape(g, 128, tb)
            .transpose(1, 0, 2)
            .reshape(128, g * tb)
        )
    return out


def _unpack_blocked(aP, cap, blocks):
    g = aP.shape[1] // cap
    out = np.empty((g * 128, cap), aP.dtype)
    for t0, tb in blocks:
        blk = aP[:, g * t0 : g * (t0 + tb)].reshape(128, g, tb)
        out[:, t0 : t0 + tb] = blk.transpose(1, 0, 2).reshape(g * 128, tb)
    return out


def _prep_weight(w):
    k, m = w.shape
    return np.ascontiguousarray(
        w.reshape(k // 128, 128, m // 128, 128).transpose(2, 1, 0, 3).reshape(m, k),
        dtype=np.float32,
    )


def kernel(x, Wr, W1, W2, _trace=False):
    x = np.asarray(x)
    Wr = np.asarray(Wr)
    W1 = np.asarray(W1)
    W2 = np.asarray(W2)
    B, T, C = x.shape
    N = B * T
    xf = np.ascontiguousarray(x.reshape(N, C), dtype=np.float32)

    # Router in float64 (matches jax f32 top_k selections; verified).
    logits = xf.astype(np.float64) @ Wr.astype(np.float64)
    logits -= logits.max(axis=-1, keepdims=True)
    p = np.exp(logits)
    p /= p.sum(axis=-1, keepdims=True)
    idx = np.argsort(-p, axis=-1, kind="stable")[:, :TOP_K]  # [N, K]
    wts = np.take_along_axis(p, idx, axis=-1)  # [N, K]

    # Dispatch list sorted by expert.
    flat_e = idx.ravel()
    order = np.argsort(flat_e, kind="stable")
    tok_of_pair = np.repeat(np.arange(N), TOP_K)[order]
    w_of_pair = wts.ravel()[order]
    counts = np.bincount(flat_e, minlength=NUM_EXPERTS)
    starts = np.concatenate([[0], np.cumsum(counts)[:-1]])

    cap = int(max(256, -(-int(counts.max()) // 128) * 128))
    if cap == 384:
        cap = 512
    blocks = _token_blocks(cap)

    in_maps = []
    toks_per_e = []
    for e in range(NUM_EXPERTS):
        s, c = int(starts[e]), int(counts[e])
        toks = tok_of_pair[s : s + c]
        toks_per_e.append(toks)
        ws = w_of_pair[s : s + c].astype(np.float32)
        xg = xf[toks] * np.sqrt(ws)[:, None]
        xTe = np.zeros((C, cap), np.float32)
        xTe[:, :c] = xg.T
        in_maps.append(
            {
                "xP": _pack_blocked(xTe, cap, blocks),
                "w1": _prep_weight(W1[e]),
                "w2": _prep_weight(W2[e]),
            }
        )

    nc = _get_program(cap)
    res = run_bass_kernel_spmd(nc, in_maps, core_ids=list(range(N_CORES)), trace=_trace)

    out = np.zeros((N, C), np.float32)
    for e in range(NUM_EXPERTS):
        c = int(counts[e])
        if c:
            yT = _unpack_blocked(res.results[e]["yP"], cap, blocks)
            out[toks_per_e[e]] += yT[:, :c].T
    if _trace:
        kernel._last_exec_time_ns = res.exec_time_ns
    return out.reshape(B, T, C)
